# revision 2
# baseline (speedup 1.0000x reference)
"""MoE (16 experts, top-1 gate, D=H=768) Trainium2 kernel.

Strategy (expert-parallel, per the sharding hint):
  - Host computes the gate (logits argmax) — this IS the dispatch step that
    decides the sharding: tokens are routed to the core owning their expert.
  - 16 experts are sharded 2-per-core across the 8 NeuronCores. Experts are
    sorted by routed-token count: the 8 largest go in slot 0 (capacity C0),
    the 8 smallest in slot 1 (capacity C1 <= C0), so every core does the
    identical padded work and padding waste is minimized. Capacities are
    multiples of 32 (matmul free dim has no 128 constraint).
  - Each core runs the two-GEMM MLP (x @ W1.T -> GELU -> @ W2.T) for its two
    experts over its routed tokens, padded to the slot capacity.
  - Host scatters per-token outputs back to the full [B, N, D] tensor.

Device kernel details (v2 — tuned from the NTFF trace of v1):
  - The PE clock ramps (0.65 -> 1.2 -> 2.4 GHz) over ~5us of continuous
    execution. A run of dependency-free warmup matmuls on a zeroed SBUF
    tile fills the otherwise idle DMA-wait head so real matmuls start at
    (or near) full clock.
  - MM1 iterates d-chunk OUTER / h-chunk inner with 6 live PSUM banks, so
    compute starts after only the first (w1 d-chunk, x d-chunk) pieces
    land and streams with no DMA bubbles. MM2 accumulates over h into 2
    rotating PSUM banks (6 + 2 = all 8 banks).
  - w2 is relaid out host-side as [128, DC, HC, 128] so its DMA pieces
    arrive in the order MM2 consumes them (d-chunk major).
  - Three HWDGE rings run in parallel: SP (slot0 weights + slot0 y out),
    ACT (x pieces, slot1 y out), POOL/gpsimd (biases, slot1 weights,
    slot1 final y pieces). Biases ride one combined [128, 24] f32 DMA.
  - Matmul operands are fp16 (full PE rate, fp32 PSUM accumulation,
    ~4e-4 end-to-end rel err). y outputs are written as fp16 (+~2e-4) to
    halve the output drain; the host casts back to fp32.
"""

import json

import ml_dtypes
import numpy as np

import concourse.bass as bass
import concourse.mybir as mybir
import concourse.tile as tile
from concourse.bass_utils import run_bass_kernel_spmd

E = 16          # experts
D = 768         # d_model
H = 768         # d_hidden
NCORES = 8
EPC = E // NCORES   # experts (slots) per core = 2
DC = D // 128       # 6 d-chunks
HC = H // 128       # 6 h-chunks
BCOL = HC + DC      # bias columns per slot in the combined bias tile

MM_DTYPE = "f16"   # "f16" | "bf16"
NWARM = 5          # PE clock warmup matmuls (512 cols each, no deps)

F32 = mybir.dt.float32
F16 = mybir.dt.float16


def _mm_dt():
    if MM_DTYPE == "f16":
        return mybir.dt.float16, np.float16
    return mybir.dt.bfloat16, ml_dtypes.bfloat16


def _split_multi_waits(nc):
    """Walrus (this image's build) rejects >1 sem-wait on one instruction
    ("Too many sync wait commands" on the TileContext-exit Drain). Move
    excess waits onto a chain of same-engine NoOps directly before the
    instruction — the sequencer runs them in program order, so the
    happens-after relation is preserved exactly."""
    bir = json.loads(nc.to_json_bytes())
    nid = 0
    for fn in bir["functions"]:
        for blk in fn["blocks"]:
            out = []
            for ins in blk["instructions"]:
                si = ins.get("sync_info")
                waits = (si or {}).get("on_wait") or []
                if len(waits) > 1:
                    for w in waits[:-1]:
                        nid += 1
                        out.append({
                            "debug": ins.get("debug", 0),
                            "name": f"I-waitfix{nid}",
                            "opcode": "NoOp",
                            "engine": ins["engine"],
                            "ins": [],
                            "outs": [],
                            "sync_info": {"on_update": [], "on_wait": [w]},
                        })
                    si["on_wait"] = waits[-1:]
                out.append(ins)
            blk["instructions"] = out
    data = json.dumps(bir).encode()
    nc.to_json_bytes = lambda: data
    return nc


def _chunking(C):
    chunks = []
    c0 = 0
    while c0 < C:
        cw = min(512, C - c0)
        chunks.append((c0, cw))
        c0 += cw
    return chunks


def _build(C0, C1):
    """Per-core SPMD kernel: slot 0 with token capacity C0, slot 1 with C1
    (both multiples of 32). Token dim in chunks of <=512 (PSUM bank limit
    for fp32 accumulation)."""
    caps = [C0, C1]
    slot_chunks = [_chunking(C) for C in caps]

    MMDT, _ = _mm_dt()

    nc = bass.Bass("TRN2", target_bir_lowering=False, debug=False,
                   num_devices=NCORES)
    # Layouts match the SBUF tiles exactly (partition-major) so every DMA is
    # a large contiguous burst.
    xts_d = [nc.dram_tensor(f"xt{s}", [128, DC, caps[s]], MMDT,
                            kind="ExternalInput") for s in range(EPC)]
    yts_d = [nc.dram_tensor(f"yt{s}", [128, DC, caps[s]], F16,
                            kind="ExternalOutput") for s in range(EPC)]
    w1t = nc.dram_tensor("w1t", [EPC, 128, DC, H], MMDT, kind="ExternalInput")
    # w2 d-chunk major: [e, i, dcb, hc, dd] = W2[e, dcb*128+dd, hc*128+i]
    w2t = nc.dram_tensor("w2t", [EPC, 128, DC, HC, 128], MMDT,
                         kind="ExternalInput")
    # combined biases: per slot, HC cols of b1 then DC cols of b2
    bct = nc.dram_tensor("bct", [128, EPC * BCOL], F32, kind="ExternalInput")

    GELU = mybir.ActivationFunctionType.Gelu

    with tile.TileContext(nc) as tc:
        with (
            tc.tile_pool(name="xp", bufs=1) as xp,
            tc.tile_pool(name="wp", bufs=1) as wp,
            tc.tile_pool(name="gp", bufs=2) as gp,
            tc.tile_pool(name="yp", bufs=3) as yp,
            tc.tile_pool(name="bp", bufs=1) as bp,
            tc.tile_pool(name="pm", bufs=6, space="PSUM") as pm,
            tc.tile_pool(name="p2", bufs=2, space="PSUM") as p2,
        ):
            # ---- phase 0: PE clock warmup. The PE ramps 0.65 -> 1.2 ->
            # 2.4 GHz with ~5us of continuous execution; these matmuls have
            # no DMA deps and run during the otherwise-idle head so real
            # matmuls start near full clock.
            wu = wp.tile([128, 640], MMDT, tag="wu", name="wu")
            nc.vector.memset(wu[:, :], 0.0)
            for i in range(NWARM):
                pw = p2.tile([128, 512], F32, tag="p2", name=f"pwu_{i}")
                nc.tensor.matmul(pw[:, :], wu[:, 0:128], wu[:, 128:640],
                                 start=True, stop=True)

            # ---- phase 1: issue ALL input DMAs. No compute-dependent wait
            # ever enters any HWDGE ring, so all three stream continuously.
            # SP ring: slot0 weights in consume-order pieces.
            # ACT ring: x pieces (slot0 in d-chunk pieces, slot1 whole).
            # POOL ring: biases, then slot1 weights.
            w1s_t, w2s_t, xcs_t = [], [], []
            for s in range(EPC):
                w1s_t.append(wp.tile([128, DC, H], MMDT, tag=f"w1_{s}",
                                     name=f"w1s_{s}"))
                w2s_t.append(wp.tile([128, DC, HC, 128], MMDT, tag=f"w2_{s}",
                                     name=f"w2s_{s}"))
                xcs_t.append([xp.tile([128, DC, 512], MMDT, tag=f"x_{s}_{ci}",
                                      name=f"xc_{s}_{ci}")
                              for ci in range(len(slot_chunks[s]))])

            # slot0 w1 in 3 progressive pieces (dc 0 | 1-2 | 3-5), then w2
            # in 3 pieces (dcb 0 | 1-2 | 3-5): first matmul after ~0.2 MB.
            for dl, dh in ((0, 1), (1, 3), (3, 6)):
                nc.sync.dma_start(w1s_t[0][:, dl:dh], w1t.ap()[0, :, dl:dh])
            for dl, dh in ((0, 1), (1, 3), (3, 6)):
                nc.sync.dma_start(w2s_t[0][:, dl:dh], w2t.ap()[0, :, dl:dh])

            # x: slot0 in 3 pieces matching MM1's dc-outer consumption.
            for ci, (c0, cw) in enumerate(slot_chunks[0]):
                for dl, dh in ((0, 1), (1, 3), (3, 6)):
                    nc.scalar.dma_start(xcs_t[0][ci][:, dl:dh, :cw],
                                        xts_d[0].ap()[:, dl:dh, c0:c0 + cw])
            for ci, (c0, cw) in enumerate(slot_chunks[1]):
                nc.scalar.dma_start(xcs_t[1][ci][:, :, :cw],
                                    xts_d[1].ap()[:, :, c0:c0 + cw])

            # POOL ring: biases (one small DMA), then slot1 weights whole.
            bc = bp.tile([128, EPC * BCOL], F32, tag="bc", name="bc")
            nc.gpsimd.dma_start(bc[:, :], bct.ap())
            nc.gpsimd.dma_start(w1s_t[1][:, :, :], w1t.ap()[1])
            nc.gpsimd.dma_start(w2s_t[1][:, :, :], w2t.ap()[1])

            # ---- phase 2: compute
            for s in range(EPC):
                chunks = slot_chunks[s]
                w1s, w2s, xcs = w1s_t[s], w2s_t[s], xcs_t[s]
                last_slot = (s == EPC - 1)
                for ci, (c0, cw) in enumerate(chunks):
                    xc = xcs[ci]
                    last_chunk = last_slot and (ci == len(chunks) - 1)
                    # MM1, d-chunk outer: 6 live PSUM banks, streams as
                    # pieces arrive.
                    ms = [pm.tile([128, 512], F32, tag="m",
                                  name=f"m_{s}_{ci}_{hc}")
                          for hc in range(HC)]
                    for dc in range(DC):
                        for hc in range(HC):
                            nc.tensor.matmul(
                                ms[hc][:, :cw],
                                w1s[:, dc, hc * 128:(hc + 1) * 128],
                                xc[:, dc, :cw],
                                start=(dc == 0), stop=(dc == DC - 1),
                            )
                    gc = gp.tile([128, HC, 512], MMDT, tag="g",
                                 name=f"gc_{s}_{ci}")
                    for hc in range(HC):
                        nc.scalar.activation(
                            gc[:, hc, :cw], ms[hc][:, :cw], GELU,
                            bias=bc[:, s * BCOL + hc:s * BCOL + hc + 1],
                            scale=1.0)
                    # MM2: accumulate over h into 2 rotating banks; outputs
                    # grouped 3 d-chunks per DMA, except the very last
                    # group which flushes per-d-chunk on alternating rings
                    # so the tail pipeline drains early.
                    for g2 in range(2):
                        dl, dh = 3 * g2, 3 * (g2 + 1)
                        split_out = last_chunk and g2 == 1
                        yc = yp.tile([128, 3, 512], F16, tag="y",
                                     name=f"yc_{s}_{ci}_{g2}")
                        for dc in range(dl, dh):
                            ps2 = p2.tile([128, 512], F32, tag="p2",
                                          name=f"p2_{s}_{ci}_{dc}")
                            for hc in range(HC):
                                nc.tensor.matmul(
                                    ps2[:, :cw],
                                    w2s[:, dc, hc, :],
                                    gc[:, hc, :cw],
                                    start=(hc == 0), stop=(hc == HC - 1),
                                )
                            nc.vector.tensor_scalar_add(
                                yc[:, dc - dl, :cw], ps2[:, :cw],
                                bc[:, s * BCOL + HC + dc:
                                   s * BCOL + HC + dc + 1])
                            if split_out:
                                eng = (nc.scalar, nc.gpsimd, nc.scalar)[dc - dl]
                                eng.dma_start(
                                    yts_d[s].ap()[:, dc, c0:c0 + cw],
                                    yc[:, dc - dl, :cw])
                        if not split_out:
                            eng = nc.sync if s == 0 else nc.scalar
                            eng.dma_start(
                                yts_d[s].ap()[:, dl:dh, c0:c0 + cw],
                                yc[:, :, :cw])

    return _split_multi_waits(nc)


_NC_CACHE = {}


def _get_nc(C0, C1):
    key = (C0, C1, MM_DTYPE, NWARM)
    nc = _NC_CACHE.get(key)
    if nc is None:
        nc = _build(C0, C1)
        _NC_CACHE[key] = nc
    return nc


def _cap(n):
    return int(max(64, -(-int(n) // 32) * 32))


def kernel(x, W1, b1, W2, b2, Wg, bg):
    x = np.ascontiguousarray(np.asarray(x, dtype=np.float32))
    W1 = np.asarray(W1, dtype=np.float32)
    b1 = np.asarray(b1, dtype=np.float32)
    W2 = np.asarray(W2, dtype=np.float32)
    b2 = np.asarray(b2, dtype=np.float32)
    Wg = np.asarray(Wg, dtype=np.float32)
    bg = np.asarray(bg, dtype=np.float32)

    B, N, Dx = x.shape
    assert Dx == D and W1.shape == (E, H, D)
    T = B * N
    t = x.reshape(T, D)

    # --- gate / dispatch (host): this decides the sharding ---
    logits = t @ Wg.T + bg
    idx = np.argmax(logits, axis=1)

    counts = np.bincount(idx, minlength=E)
    # slot 0 <- 8 largest experts, slot 1 <- 8 smallest
    order = np.argsort(-counts, kind="stable")
    slot_experts = [order[:NCORES], order[NCORES:]]
    C0 = _cap(counts[slot_experts[0]].max())
    C1 = _cap(counts[slot_experts[1]].max())
    caps = [C0, C1]
    nc = _get_nc(C0, C1)
    _, npdt = _mm_dt()

    tok_ids = [np.nonzero(idx == e)[0] for e in range(E)]

    # --- host-side layout prep ---
    t_mm = t.astype(npdt)
    # w1t[e, i, dc, h] = W1[e, h, dc*128+i] (partition-major, chunk, col)
    w1t_all = np.ascontiguousarray(
        W1.astype(npdt).transpose(0, 2, 1).reshape(E, DC, 128, H)
        .transpose(0, 2, 1, 3))
    # w2t[e, i, dcb, hc, dd] = W2[e, dcb*128+dd, hc*128+i]
    w2t_all = np.ascontiguousarray(
        W2.astype(npdt).reshape(E, DC, 128, HC, 128).transpose(0, 4, 1, 3, 2))
    # bct[i, s*BCOL + hc] = b1[e_s, hc*128+i]; [..., HC + dc] = b2[e_s, ...]
    b1c_all = b1.reshape(E, HC, 128).transpose(0, 2, 1)
    b2c_all = b2.reshape(E, DC, 128).transpose(0, 2, 1)

    in_maps = []
    for c in range(NCORES):
        experts = [int(slot_experts[s][c]) for s in range(EPC)]
        bct = np.empty((128, EPC * BCOL), np.float32)
        for s in range(EPC):
            e = experts[s]
            bct[:, s * BCOL:s * BCOL + HC] = b1c_all[e]
            bct[:, s * BCOL + HC:(s + 1) * BCOL] = b2c_all[e]
        m = {
            "w1t": np.ascontiguousarray(w1t_all[experts]),
            "w2t": np.ascontiguousarray(w2t_all[experts]),
            "bct": bct,
        }
        for s in range(EPC):
            C = caps[s]
            xts = np.zeros((128, DC, C), npdt)
            ids = tok_ids[experts[s]]
            n = len(ids)
            if n:
                xts[:, :, :n] = (
                    t_mm[ids].T.reshape(DC, 128, n).transpose(1, 0, 2))
            m[f"xt{s}"] = xts
        in_maps.append(m)

    res = run_bass_kernel_spmd(nc, in_maps, core_ids=list(range(NCORES)))

    out = np.empty((T, D), np.float32)
    for c in range(NCORES):
        for s in range(EPC):
            e = int(slot_experts[s][c])
            ids = tok_ids[e]
            n = len(ids)
            if n:
                yt = res.results[c][f"yt{s}"]  # [128, DC, C] f16
                out[ids] = (yt.transpose(1, 0, 2).reshape(D, caps[s])[:, :n]
                            .astype(np.float32).T)
    return out.reshape(B, N, D)


# revision 5
# speedup vs baseline: 1.0962x; 1.0962x over previous
"""MoE (16 experts, top-1 gate, D=H=768) Trainium2 kernel.

Strategy (expert-parallel, per the sharding hint):
  - Host computes the gate (logits argmax) — this IS the dispatch step that
    decides the sharding: tokens are routed to the core owning their expert.
  - 16 experts are sharded 2-per-core across the 8 NeuronCores. Experts are
    sorted by routed-token count: the 8 largest go in slot 0 (capacity C0),
    the 8 smallest in slot 1 (capacity C1 <= C0), so every core does the
    identical padded work and padding waste is minimized. Capacities are
    multiples of 32 (matmul free dim has no 128 constraint).
  - Each core runs the two-GEMM MLP (x @ W1.T -> GELU -> @ W2.T) for its two
    experts over its routed tokens, padded to the slot capacity.
  - Host scatters per-token outputs back to the full [B, N, D] tensor.

Device kernel details (v2 — tuned from the NTFF trace of v1):
  - The PE clock ramps (0.65 -> 1.2 -> 2.4 GHz) over ~5us of continuous
    execution. A run of dependency-free warmup matmuls on a zeroed SBUF
    tile fills the otherwise idle DMA-wait head so real matmuls start at
    (or near) full clock.
  - MM1 iterates d-chunk OUTER / h-chunk inner with 6 live PSUM banks, so
    compute starts after only the first (w1 d-chunk, x d-chunk) pieces
    land and streams with no DMA bubbles. MM2 accumulates over h into 2
    rotating PSUM banks (6 + 2 = all 8 banks).
  - w2 is relaid out host-side as [128, DC, HC, 128] so its DMA pieces
    arrive in the order MM2 consumes them (d-chunk major).
  - Three HWDGE rings run in parallel: SP (slot0 weights + slot0 y out),
    ACT (x pieces, slot1 y out), POOL/gpsimd (biases, slot1 weights,
    slot1 final y pieces). Biases ride one combined [128, 24] f32 DMA.
  - Matmul operands are fp16 (full PE rate, fp32 PSUM accumulation,
    ~4e-4 end-to-end rel err). y outputs are written as fp16 (+~2e-4) to
    halve the output drain; the host casts back to fp32.
"""

import json

import ml_dtypes
import numpy as np

import concourse.bass as bass
import concourse.mybir as mybir
import concourse.tile as tile
from concourse.bass_utils import run_bass_kernel_spmd

E = 16          # experts
D = 768         # d_model
H = 768         # d_hidden
NCORES = 8
EPC = E // NCORES   # experts (slots) per core = 2
DC = D // 128       # 6 d-chunks
HC = H // 128       # 6 h-chunks
BCOL = HC + DC      # bias columns per slot in the combined bias tile

MM_DTYPE = "f16"   # "f16" | "bf16"
NWARM = 5          # PE clock warmup matmuls (512 cols each, no deps)

F32 = mybir.dt.float32
F16 = mybir.dt.float16


def _mm_dt():
    if MM_DTYPE == "f16":
        return mybir.dt.float16, np.float16
    return mybir.dt.bfloat16, ml_dtypes.bfloat16


def _split_multi_waits(nc):
    """Walrus (this image's build) rejects >1 sem-wait on one instruction
    ("Too many sync wait commands" on the TileContext-exit Drain). Move
    excess waits onto a chain of same-engine NoOps directly before the
    instruction — the sequencer runs them in program order, so the
    happens-after relation is preserved exactly."""
    bir = json.loads(nc.to_json_bytes())
    nid = 0
    for fn in bir["functions"]:
        for blk in fn["blocks"]:
            out = []
            for ins in blk["instructions"]:
                si = ins.get("sync_info")
                waits = (si or {}).get("on_wait") or []
                if len(waits) > 1:
                    for w in waits[:-1]:
                        nid += 1
                        out.append({
                            "debug": ins.get("debug", 0),
                            "name": f"I-waitfix{nid}",
                            "opcode": "NoOp",
                            "engine": ins["engine"],
                            "ins": [],
                            "outs": [],
                            "sync_info": {"on_update": [], "on_wait": [w]},
                        })
                    si["on_wait"] = waits[-1:]
                out.append(ins)
            blk["instructions"] = out
    data = json.dumps(bir).encode()
    nc.to_json_bytes = lambda: data
    return nc


def _chunking(C):
    chunks = []
    c0 = 0
    while c0 < C:
        cw = min(512, C - c0)
        chunks.append((c0, cw))
        c0 += cw
    return chunks


def _build(C0, C1):
    """Per-core SPMD kernel: slot 0 with token capacity C0, slot 1 with C1
    (both multiples of 32). Token dim in chunks of <=512 (PSUM bank limit
    for fp32 accumulation)."""
    caps = [C0, C1]
    slot_chunks = [_chunking(C) for C in caps]

    MMDT, _ = _mm_dt()

    nc = bass.Bass("TRN2", target_bir_lowering=False, debug=False,
                   num_devices=NCORES)
    # Layouts match the SBUF tiles exactly (partition-major) so every DMA is
    # a large contiguous burst.
    xts_d = [nc.dram_tensor(f"xt{s}", [128, DC, caps[s]], MMDT,
                            kind="ExternalInput") for s in range(EPC)]
    yts_d = [nc.dram_tensor(f"yt{s}", [128, DC, caps[s]], F16,
                            kind="ExternalOutput") for s in range(EPC)]
    w1t = nc.dram_tensor("w1t", [EPC, 128, DC, H], MMDT, kind="ExternalInput")
    # w2 d-chunk major: [e, i, dcb, hc, dd] = W2[e, dcb*128+dd, hc*128+i]
    w2t = nc.dram_tensor("w2t", [EPC, 128, DC, HC, 128], MMDT,
                         kind="ExternalInput")
    # combined biases: per slot, HC cols of b1 then DC cols of b2
    bct = nc.dram_tensor("bct", [128, EPC * BCOL], F32, kind="ExternalInput")

    GELU = mybir.ActivationFunctionType.Gelu

    with tile.TileContext(nc) as tc:
        with (
            tc.tile_pool(name="xp", bufs=1) as xp,
            tc.tile_pool(name="wp", bufs=1) as wp,
            tc.tile_pool(name="gp", bufs=2) as gp,
            tc.tile_pool(name="yp", bufs=3) as yp,
            tc.tile_pool(name="bp", bufs=1) as bp,
            tc.tile_pool(name="pm", bufs=6, space="PSUM") as pm,
            tc.tile_pool(name="p2", bufs=2, space="PSUM") as p2,
        ):
            # ---- phase 0: PE clock warmup. The PE ramps 0.65 -> 1.2 ->
            # 2.4 GHz with ~5us of continuous execution; these matmuls have
            # no DMA deps and run during the otherwise-idle head so real
            # matmuls start near full clock.
            wu = wp.tile([128, 640], MMDT, tag="wu", name="wu")
            nc.vector.memset(wu[:, :], 0.0)
            for i in range(NWARM):
                pw = p2.tile([128, 512], F32, tag="p2", name=f"pwu_{i}")
                nc.tensor.matmul(pw[:, :], wu[:, 0:128], wu[:, 128:640],
                                 start=True, stop=True)

            # ---- phase 1: issue ALL input DMAs. No compute-dependent wait
            # ever enters any HWDGE ring. HBM bandwidth is SHARED across the
            # rings (~450 GB/s/core aggregate), so later-needed tensors must
            # ride BEHIND earlier-needed ones on the same ring rather than
            # on a parallel ring (parallel rings steal bandwidth from the
            # critical path — measured +11us when slot1 weights ran on
            # their own ring during the slot0 window).
            # SP ring:  w1s0 pieces | w2s0 pieces | w1s1 halves (consume order)
            # ACT ring: x0 pieces | x1 | w2s1, then y outputs
            # POOL ring: biases (tiny), then tail y pieces
            w1s_t, w2s_t, xcs_t = [], [], []
            for s in range(EPC):
                w1s_t.append(wp.tile([128, DC, H], MMDT, tag=f"w1_{s}",
                                     name=f"w1s_{s}"))
                w2s_t.append(wp.tile([128, DC, HC, 128], MMDT, tag=f"w2_{s}",
                                     name=f"w2s_{s}"))
                xcs_t.append([xp.tile([128, DC, cw], MMDT, tag=f"x_{s}_{ci}",
                                      name=f"xc_{s}_{ci}")
                              for ci, (c0, cw) in enumerate(slot_chunks[s])])

            # slot0 w1 in 3 progressive pieces (dc 0 | 1-2 | 3-5), then w2
            # in 3 pieces (dcb 0 | 1-2 | 3-5), then slot1 w1 in halves:
            # first matmul after ~0.2 MB, every later piece lands before use.
            for dl, dh in ((0, 1), (1, 3), (3, 6)):
                nc.sync.dma_start(w1s_t[0][:, dl:dh], w1t.ap()[0, :, dl:dh])
            for dl, dh in ((0, 1), (1, 3), (3, 6)):
                nc.sync.dma_start(w2s_t[0][:, dl:dh], w2t.ap()[0, :, dl:dh])
            for dl, dh in ((0, 3), (3, 6)):
                nc.sync.dma_start(w1s_t[1][:, dl:dh], w1t.ap()[1, :, dl:dh])

            # x: slot0 in 3 pieces matching MM1's dc-outer consumption,
            # slot1 whole, then slot1 w2 whole.
            for ci, (c0, cw) in enumerate(slot_chunks[0]):
                for dl, dh in ((0, 1), (1, 3), (3, 6)):
                    nc.scalar.dma_start(xcs_t[0][ci][:, dl:dh, :],
                                        xts_d[0].ap()[:, dl:dh, c0:c0 + cw])
            for ci, (c0, cw) in enumerate(slot_chunks[1]):
                nc.scalar.dma_start(xcs_t[1][ci][:, :, :],
                                    xts_d[1].ap()[:, :, c0:c0 + cw])
            nc.scalar.dma_start(w2s_t[1][:, :, :], w2t.ap()[1])

            # POOL ring: biases (one small DMA).
            bc = bp.tile([128, EPC * BCOL], F32, tag="bc", name="bc")
            nc.gpsimd.dma_start(bc[:, :], bct.ap())

            # ---- phase 2: compute
            for s in range(EPC):
                chunks = slot_chunks[s]
                w1s, w2s, xcs = w1s_t[s], w2s_t[s], xcs_t[s]
                last_slot = (s == EPC - 1)
                for ci, (c0, cw) in enumerate(chunks):
                    xc = xcs[ci]
                    last_chunk = last_slot and (ci == len(chunks) - 1)
                    # MM1, d-chunk outer: 6 live PSUM banks, streams as
                    # pieces arrive.
                    ms = [pm.tile([128, 512], F32, tag="m",
                                  name=f"m_{s}_{ci}_{hc}")
                          for hc in range(HC)]
                    for dc in range(DC):
                        for hc in range(HC):
                            nc.tensor.matmul(
                                ms[hc][:, :cw],
                                w1s[:, dc, hc * 128:(hc + 1) * 128],
                                xc[:, dc, :],
                                start=(dc == 0), stop=(dc == DC - 1),
                            )
                    gc = gp.tile([128, HC, 512], MMDT, tag="g",
                                 name=f"gc_{s}_{ci}")
                    for hc in range(HC):
                        nc.scalar.activation(
                            gc[:, hc, :cw], ms[hc][:, :cw], GELU,
                            bias=bc[:, s * BCOL + hc:s * BCOL + hc + 1],
                            scale=1.0)
                    # MM2: accumulate over h into 2 rotating banks; outputs
                    # grouped 3 d-chunks per DMA, except the very last
                    # group which flushes per-d-chunk on alternating rings
                    # so the tail pipeline drains early.
                    for g2 in range(2):
                        dl, dh = 3 * g2, 3 * (g2 + 1)
                        split_out = last_chunk and g2 == 1
                        yc = yp.tile([128, 3, cw], F16, tag="y",
                                     name=f"yc_{s}_{ci}_{g2}")
                        for dc in range(dl, dh):
                            ps2 = p2.tile([128, 512], F32, tag="p2",
                                          name=f"p2_{s}_{ci}_{dc}")
                            for hc in range(HC):
                                nc.tensor.matmul(
                                    ps2[:, :cw],
                                    w2s[:, dc, hc, :],
                                    gc[:, hc, :cw],
                                    start=(hc == 0), stop=(hc == HC - 1),
                                )
                            nc.vector.tensor_scalar_add(
                                yc[:, dc - dl, :], ps2[:, :cw],
                                bc[:, s * BCOL + HC + dc:
                                   s * BCOL + HC + dc + 1])
                            if split_out:
                                eng = (nc.scalar, nc.gpsimd, nc.scalar)[dc - dl]
                                eng.dma_start(
                                    yts_d[s].ap()[:, dc, c0:c0 + cw],
                                    yc[:, dc - dl, :])
                        if not split_out:
                            nc.scalar.dma_start(
                                yts_d[s].ap()[:, dl:dh, c0:c0 + cw],
                                yc[:, :, :])

    return _split_multi_waits(nc)


_NC_CACHE = {}


def _get_nc(C0, C1):
    key = (C0, C1, MM_DTYPE, NWARM)
    nc = _NC_CACHE.get(key)
    if nc is None:
        nc = _build(C0, C1)
        _NC_CACHE[key] = nc
    return nc


def _cap(n):
    return int(max(64, -(-int(n) // 32) * 32))


def kernel(x, W1, b1, W2, b2, Wg, bg):
    x = np.ascontiguousarray(np.asarray(x, dtype=np.float32))
    W1 = np.asarray(W1, dtype=np.float32)
    b1 = np.asarray(b1, dtype=np.float32)
    W2 = np.asarray(W2, dtype=np.float32)
    b2 = np.asarray(b2, dtype=np.float32)
    Wg = np.asarray(Wg, dtype=np.float32)
    bg = np.asarray(bg, dtype=np.float32)

    B, N, Dx = x.shape
    assert Dx == D and W1.shape == (E, H, D)
    T = B * N
    t = x.reshape(T, D)

    # --- gate / dispatch (host): this decides the sharding ---
    logits = t @ Wg.T + bg
    idx = np.argmax(logits, axis=1)

    counts = np.bincount(idx, minlength=E)
    # slot 0 <- 8 largest experts, slot 1 <- 8 smallest
    order = np.argsort(-counts, kind="stable")
    slot_experts = [order[:NCORES], order[NCORES:]]
    C0 = _cap(counts[slot_experts[0]].max())
    C1 = _cap(counts[slot_experts[1]].max())
    caps = [C0, C1]
    nc = _get_nc(C0, C1)
    _, npdt = _mm_dt()

    tok_ids = [np.nonzero(idx == e)[0] for e in range(E)]

    # --- host-side layout prep ---
    t_mm = t.astype(npdt)
    # w1t[e, i, dc, h] = W1[e, h, dc*128+i] (partition-major, chunk, col)
    w1t_all = np.ascontiguousarray(
        W1.astype(npdt).transpose(0, 2, 1).reshape(E, DC, 128, H)
        .transpose(0, 2, 1, 3))
    # w2t[e, i, dcb, hc, dd] = W2[e, dcb*128+dd, hc*128+i]
    w2t_all = np.ascontiguousarray(
        W2.astype(npdt).reshape(E, DC, 128, HC, 128).transpose(0, 4, 1, 3, 2))
    # bct[i, s*BCOL + hc] = b1[e_s, hc*128+i]; [..., HC + dc] = b2[e_s, ...]
    b1c_all = b1.reshape(E, HC, 128).transpose(0, 2, 1)
    b2c_all = b2.reshape(E, DC, 128).transpose(0, 2, 1)

    in_maps = []
    for c in range(NCORES):
        experts = [int(slot_experts[s][c]) for s in range(EPC)]
        bct = np.empty((128, EPC * BCOL), np.float32)
        for s in range(EPC):
            e = experts[s]
            bct[:, s * BCOL:s * BCOL + HC] = b1c_all[e]
            bct[:, s * BCOL + HC:(s + 1) * BCOL] = b2c_all[e]
        m = {
            "w1t": np.ascontiguousarray(w1t_all[experts]),
            "w2t": np.ascontiguousarray(w2t_all[experts]),
            "bct": bct,
        }
        for s in range(EPC):
            C = caps[s]
            xts = np.zeros((128, DC, C), npdt)
            ids = tok_ids[experts[s]]
            n = len(ids)
            if n:
                xts[:, :, :n] = (
                    t_mm[ids].T.reshape(DC, 128, n).transpose(1, 0, 2))
            m[f"xt{s}"] = xts
        in_maps.append(m)

    res = run_bass_kernel_spmd(nc, in_maps, core_ids=list(range(NCORES)))

    out = np.empty((T, D), np.float32)
    for c in range(NCORES):
        for s in range(EPC):
            e = int(slot_experts[s][c])
            ids = tok_ids[e]
            n = len(ids)
            if n:
                yt = res.results[c][f"yt{s}"]  # [128, DC, C] f16
                out[ids] = (yt.transpose(1, 0, 2).reshape(D, caps[s])[:, :n]
                            .astype(np.float32).T)
    return out.reshape(B, N, D)


# revision 7
# speedup vs baseline: 1.1510x; 1.0500x over previous
"""MoE (16 experts, top-1 gate, D=H=768) Trainium2 kernel.

Strategy (expert-parallel, per the sharding hint):
  - Host computes the gate (logits argmax) — this IS the dispatch step that
    decides the sharding: tokens are routed to the core owning their expert.
  - 16 experts are sharded 2-per-core across the 8 NeuronCores. Experts are
    sorted by routed-token count: the 8 largest go in slot 0 (capacity C0),
    the 8 smallest in slot 1 (capacity C1 <= C0), so every core does the
    identical padded work and padding waste is minimized. Capacities are
    multiples of 32 (matmul free dim has no 128 constraint).
  - Each core runs the two-GEMM MLP (x @ W1.T -> GELU -> @ W2.T) for its two
    experts over its routed tokens, padded to the slot capacity.
  - Host scatters per-token outputs back to the full [B, N, D] tensor.

Device kernel details (v2 — tuned from the NTFF trace of v1):
  - The PE clock ramps (0.65 -> 1.2 -> 2.4 GHz) over ~5us of continuous
    execution. A run of dependency-free warmup matmuls on a zeroed SBUF
    tile fills the otherwise idle DMA-wait head so real matmuls start at
    (or near) full clock.
  - MM1 iterates d-chunk OUTER / h-chunk inner with 6 live PSUM banks, so
    compute starts after only the first (w1 d-chunk, x d-chunk) pieces
    land and streams with no DMA bubbles. MM2 accumulates over h into 2
    rotating PSUM banks (6 + 2 = all 8 banks).
  - w2 is relaid out host-side as [128, DC, HC, 128] so its DMA pieces
    arrive in the order MM2 consumes them (d-chunk major).
  - Three HWDGE rings run in parallel: SP (slot0 weights + slot0 y out),
    ACT (x pieces, slot1 y out), POOL/gpsimd (biases, slot1 weights,
    slot1 final y pieces). Biases ride one combined [128, 24] f32 DMA.
  - Matmul operands are fp16 (full PE rate, fp32 PSUM accumulation,
    ~4e-4 end-to-end rel err). y outputs are written as fp16 (+~2e-4) to
    halve the output drain; the host casts back to fp32.
"""

import json

import ml_dtypes
import numpy as np

import concourse.bass as bass
import concourse.mybir as mybir
import concourse.tile as tile
from concourse.bass_utils import run_bass_kernel_spmd

E = 16          # experts
D = 768         # d_model
H = 768         # d_hidden
NCORES = 8
EPC = E // NCORES   # experts (slots) per core = 2
DC = D // 128       # 6 d-chunks
HC = H // 128       # 6 h-chunks
BCOL = HC + DC      # bias columns per slot in the combined bias tile

MM_DTYPE = "f16"   # "f16" | "bf16"
NWARM = 5          # PE clock warmup matmuls (512 cols each, no deps)

F32 = mybir.dt.float32
F16 = mybir.dt.float16


def _mm_dt():
    if MM_DTYPE == "f16":
        return mybir.dt.float16, np.float16
    return mybir.dt.bfloat16, ml_dtypes.bfloat16


def _split_multi_waits(nc):
    """Walrus (this image's build) rejects >1 sem-wait on one instruction
    ("Too many sync wait commands" on the TileContext-exit Drain). Move
    excess waits onto a chain of same-engine NoOps directly before the
    instruction — the sequencer runs them in program order, so the
    happens-after relation is preserved exactly."""
    bir = json.loads(nc.to_json_bytes())
    nid = 0
    for fn in bir["functions"]:
        for blk in fn["blocks"]:
            out = []
            for ins in blk["instructions"]:
                si = ins.get("sync_info")
                waits = (si or {}).get("on_wait") or []
                if len(waits) > 1:
                    for w in waits[:-1]:
                        nid += 1
                        out.append({
                            "debug": ins.get("debug", 0),
                            "name": f"I-waitfix{nid}",
                            "opcode": "NoOp",
                            "engine": ins["engine"],
                            "ins": [],
                            "outs": [],
                            "sync_info": {"on_update": [], "on_wait": [w]},
                        })
                    si["on_wait"] = waits[-1:]
                out.append(ins)
            blk["instructions"] = out
    data = json.dumps(bir).encode()
    nc.to_json_bytes = lambda: data
    return nc


def _chunking(C):
    chunks = []
    c0 = 0
    while c0 < C:
        cw = min(512, C - c0)
        chunks.append((c0, cw))
        c0 += cw
    return chunks


def _build(C0, C1):
    """Per-core SPMD kernel: slot 0 with token capacity C0, slot 1 with C1
    (both multiples of 32). Token dim in chunks of <=512 (PSUM bank limit
    for fp32 accumulation)."""
    caps = [C0, C1]
    slot_chunks = [_chunking(C) for C in caps]

    MMDT, _ = _mm_dt()

    nc = bass.Bass("TRN2", target_bir_lowering=False, debug=False,
                   num_devices=NCORES)
    # Layouts match the SBUF tiles exactly (partition-major) so every DMA is
    # a large contiguous burst.
    xts_d = [nc.dram_tensor(f"xt{s}", [128, DC, caps[s]], MMDT,
                            kind="ExternalInput") for s in range(EPC)]
    yts_d = [nc.dram_tensor(f"yt{s}", [128, DC, caps[s]], F16,
                            kind="ExternalOutput") for s in range(EPC)]
    w1t = nc.dram_tensor("w1t", [EPC, 128, DC, H], MMDT, kind="ExternalInput")
    # w2 d-chunk major: [e, i, dcb, hc, dd] = W2[e, dcb*128+dd, hc*128+i]
    w2t = nc.dram_tensor("w2t", [EPC, 128, DC, HC, 128], MMDT,
                         kind="ExternalInput")
    # combined biases: per slot, HC cols of b1 then DC cols of b2
    bct = nc.dram_tensor("bct", [128, EPC * BCOL], F32, kind="ExternalInput")

    GELU = mybir.ActivationFunctionType.Gelu

    with tile.TileContext(nc) as tc:
        with (
            tc.tile_pool(name="xp", bufs=1) as xp,
            tc.tile_pool(name="wp", bufs=1) as wp,
            tc.tile_pool(name="gp", bufs=2) as gp,
            tc.tile_pool(name="yp", bufs=3) as yp,
            tc.tile_pool(name="bp", bufs=1) as bp,
            tc.tile_pool(name="pm", bufs=6, space="PSUM") as pm,
            tc.tile_pool(name="p2", bufs=2, space="PSUM") as p2,
        ):
            # ---- phase 0: PE clock warmup. The PE ramps 0.65 -> 1.2 ->
            # 2.4 GHz with ~5us of continuous execution; these matmuls have
            # no DMA deps and run during the otherwise-idle head so real
            # matmuls start near full clock.
            wu = wp.tile([128, 640], MMDT, tag="wu", name="wu")
            nc.vector.memset(wu[:, :], 0.0)
            for i in range(NWARM):
                pw = p2.tile([128, 512], F32, tag="p2", name=f"pwu_{i}")
                nc.tensor.matmul(pw[:, :], wu[:, 0:128], wu[:, 128:640],
                                 start=True, stop=True)

            # ---- phase 1: issue ALL input DMAs. No compute-dependent wait
            # ever enters any HWDGE ring. HBM bandwidth is SHARED across the
            # rings (~450 GB/s/core aggregate), so later-needed tensors must
            # ride BEHIND earlier-needed ones on the same ring rather than
            # on a parallel ring (parallel rings steal bandwidth from the
            # critical path — measured +11us when slot1 weights ran on
            # their own ring during the slot0 window).
            # SP ring:  w1s0 pieces | w2s0 pieces | w1s1 halves (consume order)
            # ACT ring: x0 pieces | x1 | w2s1, then y outputs
            # POOL ring: biases (tiny), then tail y pieces
            w1s_t, w2s_t, xcs_t = [], [], []
            for s in range(EPC):
                w1s_t.append(wp.tile([128, DC, H], MMDT, tag=f"w1_{s}",
                                     name=f"w1s_{s}"))
                w2s_t.append(wp.tile([128, DC, HC, 128], MMDT, tag=f"w2_{s}",
                                     name=f"w2s_{s}"))
                xcs_t.append([xp.tile([128, DC, cw], MMDT, tag=f"x_{s}_{ci}",
                                      name=f"xc_{s}_{ci}")
                              for ci, (c0, cw) in enumerate(slot_chunks[s])])

            # slot0 w1 per-d-chunk pieces ALTERNATED across the SP and POOL
            # rings (a single ring moves small-elem pieces at only ~100-150
            # GB/s; two in parallel keep up with MM1's consumption), then
            # w2s0 halves, then w1s1 halves — strictly in consume order.
            bc = bp.tile([128, EPC * BCOL], F32, tag="bc", name="bc")
            nc.sync.dma_start(w1s_t[0][:, 0:1], w1t.ap()[0, :, 0:1])
            nc.gpsimd.dma_start(w1s_t[0][:, 1:2], w1t.ap()[0, :, 1:2])
            nc.sync.dma_start(w1s_t[0][:, 2:3], w1t.ap()[0, :, 2:3])
            nc.gpsimd.dma_start(bc[:, :], bct.ap())
            nc.sync.dma_start(w1s_t[0][:, 4:5], w1t.ap()[0, :, 4:5])
            nc.gpsimd.dma_start(w1s_t[0][:, 3:4], w1t.ap()[0, :, 3:4])
            nc.gpsimd.dma_start(w1s_t[0][:, 5:6], w1t.ap()[0, :, 5:6])
            nc.sync.dma_start(w2s_t[0][:, 0:3], w2t.ap()[0, :, 0:3])
            nc.gpsimd.dma_start(w2s_t[0][:, 3:6], w2t.ap()[0, :, 3:6])
            nc.sync.dma_start(w1s_t[1][:, 0:3], w1t.ap()[1, :, 0:3])
            nc.gpsimd.dma_start(w1s_t[1][:, 3:6], w1t.ap()[1, :, 3:6])

            # ACT ring: x slot0 in 4 pieces matching MM1's dc-outer
            # consumption, slot1 whole, then slot1 w2 whole.
            for ci, (c0, cw) in enumerate(slot_chunks[0]):
                for dl, dh in ((0, 1), (1, 2), (2, 4), (4, 6)):
                    nc.scalar.dma_start(xcs_t[0][ci][:, dl:dh, :],
                                        xts_d[0].ap()[:, dl:dh, c0:c0 + cw])
            for ci, (c0, cw) in enumerate(slot_chunks[1]):
                nc.scalar.dma_start(xcs_t[1][ci][:, :, :],
                                    xts_d[1].ap()[:, :, c0:c0 + cw])
            nc.scalar.dma_start(w2s_t[1][:, :, :], w2t.ap()[1])

            # Preload the scalar engine's GELU table (ACT_TABLE_LOAD costs
            # ~1.3us on first use; do it in the DMA-wait head, after the
            # x-piece issues so it doesn't delay them).
            gwu = gp.tile([128, 1], F32, tag="gwu", name="gwu")
            nc.scalar.activation(gwu[:, :], wu[:, 0:1], GELU, bias=0.0,
                                 scale=1.0)

            # ---- phase 2: compute
            for s in range(EPC):
                chunks = slot_chunks[s]
                w1s, w2s, xcs = w1s_t[s], w2s_t[s], xcs_t[s]
                last_slot = (s == EPC - 1)
                for ci, (c0, cw) in enumerate(chunks):
                    xc = xcs[ci]
                    last_chunk = last_slot and (ci == len(chunks) - 1)
                    streamed = (s == 0 and ci == 0)
                    gc = gp.tile([128, HC, 512], MMDT, tag="g",
                                 name=f"gc_{s}_{ci}")
                    if streamed:
                        # MM1, d-chunk outer: 6 live PSUM banks, streams as
                        # the per-d-chunk w1/x pieces arrive.
                        ms = [pm.tile([128, 512], F32, tag="m",
                                      name=f"m_{s}_{ci}_{hc}")
                              for hc in range(HC)]
                        for dc in range(DC):
                            for hc in range(HC):
                                nc.tensor.matmul(
                                    ms[hc][:, :cw],
                                    w1s[:, dc, hc * 128:(hc + 1) * 128],
                                    xc[:, dc, :],
                                    start=(dc == 0), stop=(dc == DC - 1),
                                )
                        for hc in range(HC):
                            nc.scalar.activation(
                                gc[:, hc, :cw], ms[hc][:, :cw], GELU,
                                bias=bc[:, s * BCOL + hc:s * BCOL + hc + 1],
                                scale=1.0)
                    else:
                        # Weights already resident: h-chunk outer, each PSUM
                        # completes (and GELUs) early so MM2 never waits on
                        # the activation at the slot transition.
                        for hc in range(HC):
                            mh = pm.tile([128, 512], F32, tag="m",
                                         name=f"m_{s}_{ci}_{hc}")
                            for dc in range(DC):
                                nc.tensor.matmul(
                                    mh[:, :cw],
                                    w1s[:, dc, hc * 128:(hc + 1) * 128],
                                    xc[:, dc, :],
                                    start=(dc == 0), stop=(dc == DC - 1),
                                )
                            nc.scalar.activation(
                                gc[:, hc, :cw], mh[:, :cw], GELU,
                                bias=bc[:, s * BCOL + hc:s * BCOL + hc + 1],
                                scale=1.0)
                    # MM2: accumulate over h into 2 rotating banks; outputs
                    # grouped 3 d-chunks per DMA, except the very last
                    # group which flushes per-d-chunk on alternating rings
                    # so the tail pipeline drains early.
                    for g2 in range(2):
                        dl, dh = 3 * g2, 3 * (g2 + 1)
                        split_out = last_chunk and g2 == 1
                        yc = yp.tile([128, 3, cw], F16, tag="y",
                                     name=f"yc_{s}_{ci}_{g2}")
                        for dc in range(dl, dh):
                            ps2 = p2.tile([128, 512], F32, tag="p2",
                                          name=f"p2_{s}_{ci}_{dc}")
                            for hc in range(HC):
                                nc.tensor.matmul(
                                    ps2[:, :cw],
                                    w2s[:, dc, hc, :],
                                    gc[:, hc, :cw],
                                    start=(hc == 0), stop=(hc == HC - 1),
                                )
                            nc.vector.tensor_scalar_add(
                                yc[:, dc - dl, :], ps2[:, :cw],
                                bc[:, s * BCOL + HC + dc:
                                   s * BCOL + HC + dc + 1])
                            if split_out:
                                eng = (nc.scalar, nc.gpsimd, nc.scalar)[dc - dl]
                                eng.dma_start(
                                    yts_d[s].ap()[:, dc, c0:c0 + cw],
                                    yc[:, dc - dl, :])
                        if not split_out:
                            nc.scalar.dma_start(
                                yts_d[s].ap()[:, dl:dh, c0:c0 + cw],
                                yc[:, :, :])

    return _split_multi_waits(nc)


_NC_CACHE = {}


def _get_nc(C0, C1):
    key = (C0, C1, MM_DTYPE, NWARM)
    nc = _NC_CACHE.get(key)
    if nc is None:
        nc = _build(C0, C1)
        _NC_CACHE[key] = nc
    return nc


def _cap(n):
    return int(max(64, -(-int(n) // 32) * 32))


def kernel(x, W1, b1, W2, b2, Wg, bg):
    x = np.ascontiguousarray(np.asarray(x, dtype=np.float32))
    W1 = np.asarray(W1, dtype=np.float32)
    b1 = np.asarray(b1, dtype=np.float32)
    W2 = np.asarray(W2, dtype=np.float32)
    b2 = np.asarray(b2, dtype=np.float32)
    Wg = np.asarray(Wg, dtype=np.float32)
    bg = np.asarray(bg, dtype=np.float32)

    B, N, Dx = x.shape
    assert Dx == D and W1.shape == (E, H, D)
    T = B * N
    t = x.reshape(T, D)

    # --- gate / dispatch (host): this decides the sharding ---
    logits = t @ Wg.T + bg
    idx = np.argmax(logits, axis=1)

    counts = np.bincount(idx, minlength=E)
    # slot 0 <- 8 largest experts, slot 1 <- 8 smallest
    order = np.argsort(-counts, kind="stable")
    slot_experts = [order[:NCORES], order[NCORES:]]
    C0 = _cap(counts[slot_experts[0]].max())
    C1 = _cap(counts[slot_experts[1]].max())
    caps = [C0, C1]
    nc = _get_nc(C0, C1)
    _, npdt = _mm_dt()

    tok_ids = [np.nonzero(idx == e)[0] for e in range(E)]

    # --- host-side layout prep ---
    t_mm = t.astype(npdt)
    # w1t[e, i, dc, h] = W1[e, h, dc*128+i] (partition-major, chunk, col)
    w1t_all = np.ascontiguousarray(
        W1.astype(npdt).transpose(0, 2, 1).reshape(E, DC, 128, H)
        .transpose(0, 2, 1, 3))
    # w2t[e, i, dcb, hc, dd] = W2[e, dcb*128+dd, hc*128+i]
    w2t_all = np.ascontiguousarray(
        W2.astype(npdt).reshape(E, DC, 128, HC, 128).transpose(0, 4, 1, 3, 2))
    # bct[i, s*BCOL + hc] = b1[e_s, hc*128+i]; [..., HC + dc] = b2[e_s, ...]
    b1c_all = b1.reshape(E, HC, 128).transpose(0, 2, 1)
    b2c_all = b2.reshape(E, DC, 128).transpose(0, 2, 1)

    in_maps = []
    for c in range(NCORES):
        experts = [int(slot_experts[s][c]) for s in range(EPC)]
        bct = np.empty((128, EPC * BCOL), np.float32)
        for s in range(EPC):
            e = experts[s]
            bct[:, s * BCOL:s * BCOL + HC] = b1c_all[e]
            bct[:, s * BCOL + HC:(s + 1) * BCOL] = b2c_all[e]
        m = {
            "w1t": np.ascontiguousarray(w1t_all[experts]),
            "w2t": np.ascontiguousarray(w2t_all[experts]),
            "bct": bct,
        }
        for s in range(EPC):
            C = caps[s]
            xts = np.zeros((128, DC, C), npdt)
            ids = tok_ids[experts[s]]
            n = len(ids)
            if n:
                xts[:, :, :n] = (
                    t_mm[ids].T.reshape(DC, 128, n).transpose(1, 0, 2))
            m[f"xt{s}"] = xts
        in_maps.append(m)

    res = run_bass_kernel_spmd(nc, in_maps, core_ids=list(range(NCORES)))

    out = np.empty((T, D), np.float32)
    for c in range(NCORES):
        for s in range(EPC):
            e = int(slot_experts[s][c])
            ids = tok_ids[e]
            n = len(ids)
            if n:
                yt = res.results[c][f"yt{s}"]  # [128, DC, C] f16
                out[ids] = (yt.transpose(1, 0, 2).reshape(D, caps[s])[:, :n]
                            .astype(np.float32).T)
    return out.reshape(B, N, D)


# revision 10
# speedup vs baseline: 1.1549x; 1.0033x over previous
"""MoE (16 experts, top-1 gate, D=H=768) Trainium2 kernel.

Strategy (expert-parallel, per the sharding hint):
  - Host computes the gate (logits argmax) — this IS the dispatch step that
    decides the sharding: tokens are routed to the core owning their expert.
  - 16 experts are sharded 2-per-core across the 8 NeuronCores. Experts are
    sorted by routed-token count: the 8 largest go in slot 0 (capacity C0),
    the 8 smallest in slot 1 (capacity C1 <= C0), so every core does the
    identical padded work and padding waste is minimized. Capacities are
    multiples of 32 (matmul free dim has no 128 constraint).
  - Each core runs the two-GEMM MLP (x @ W1.T -> GELU -> @ W2.T) for its two
    experts over its routed tokens, padded to the slot capacity.
  - Host scatters per-token outputs back to the full [B, N, D] tensor.

Device kernel details (v2 — tuned from the NTFF trace of v1):
  - The PE clock ramps (0.65 -> 1.2 -> 2.4 GHz) over ~5us of continuous
    execution. A run of dependency-free warmup matmuls on a zeroed SBUF
    tile fills the otherwise idle DMA-wait head so real matmuls start at
    (or near) full clock.
  - MM1 iterates d-chunk OUTER / h-chunk inner with 6 live PSUM banks, so
    compute starts after only the first (w1 d-chunk, x d-chunk) pieces
    land and streams with no DMA bubbles. MM2 accumulates over h into 2
    rotating PSUM banks (6 + 2 = all 8 banks).
  - w2 is relaid out host-side as [128, DC, HC, 128] so its DMA pieces
    arrive in the order MM2 consumes them (d-chunk major).
  - Three HWDGE rings run in parallel: SP (slot0 weights + slot0 y out),
    ACT (x pieces, slot1 y out), POOL/gpsimd (biases, slot1 weights,
    slot1 final y pieces). Biases ride one combined [128, 24] f32 DMA.
  - Matmul operands are fp16 (full PE rate, fp32 PSUM accumulation,
    ~4e-4 end-to-end rel err). y outputs are written as fp16 (+~2e-4) to
    halve the output drain; the host casts back to fp32.
"""

import json

import ml_dtypes
import numpy as np

import concourse.bass as bass
import concourse.mybir as mybir
import concourse.tile as tile
from concourse.bass_utils import run_bass_kernel_spmd

E = 16          # experts
D = 768         # d_model
H = 768         # d_hidden
NCORES = 8
EPC = E // NCORES   # experts (slots) per core = 2
DC = D // 128       # 6 d-chunks
HC = H // 128       # 6 h-chunks
BCOL = HC + DC      # bias columns per slot in the combined bias tile

MM_DTYPE = "f16"   # "f16" | "bf16"
NWARM = 5          # PE clock warmup matmuls (512 cols each, no deps)

F32 = mybir.dt.float32
F16 = mybir.dt.float16


def _mm_dt():
    if MM_DTYPE == "f16":
        return mybir.dt.float16, np.float16
    return mybir.dt.bfloat16, ml_dtypes.bfloat16


def _split_multi_waits(nc):
    """Walrus (this image's build) rejects >1 sem-wait on one instruction
    ("Too many sync wait commands" on the TileContext-exit Drain). Move
    excess waits onto a chain of same-engine NoOps directly before the
    instruction — the sequencer runs them in program order, so the
    happens-after relation is preserved exactly."""
    bir = json.loads(nc.to_json_bytes())
    nid = 0
    for fn in bir["functions"]:
        for blk in fn["blocks"]:
            out = []
            for ins in blk["instructions"]:
                si = ins.get("sync_info")
                waits = (si or {}).get("on_wait") or []
                if len(waits) > 1:
                    for w in waits[:-1]:
                        nid += 1
                        out.append({
                            "debug": ins.get("debug", 0),
                            "name": f"I-waitfix{nid}",
                            "opcode": "NoOp",
                            "engine": ins["engine"],
                            "ins": [],
                            "outs": [],
                            "sync_info": {"on_update": [], "on_wait": [w]},
                        })
                    si["on_wait"] = waits[-1:]
                out.append(ins)
            blk["instructions"] = out
    data = json.dumps(bir).encode()
    nc.to_json_bytes = lambda: data
    return nc


def _chunking(C):
    chunks = []
    c0 = 0
    while c0 < C:
        cw = min(512, C - c0)
        chunks.append((c0, cw))
        c0 += cw
    return chunks


def _build(C0, C1):
    """Per-core SPMD kernel: slot 0 with token capacity C0, slot 1 with C1
    (both multiples of 32). Token dim in chunks of <=512 (PSUM bank limit
    for fp32 accumulation)."""
    caps = [C0, C1]
    slot_chunks = [_chunking(C) for C in caps]

    MMDT, _ = _mm_dt()

    nc = bass.Bass("TRN2", target_bir_lowering=False, debug=False,
                   num_devices=NCORES)
    # Layouts match the SBUF tiles exactly (partition-major) so every DMA is
    # a large contiguous burst.
    xts_d = [nc.dram_tensor(f"xt{s}", [128, DC, caps[s]], MMDT,
                            kind="ExternalInput") for s in range(EPC)]
    yts_d = [nc.dram_tensor(f"yt{s}", [128, DC, caps[s]], F16,
                            kind="ExternalOutput") for s in range(EPC)]
    w1t = nc.dram_tensor("w1t", [EPC, 128, DC, H], MMDT, kind="ExternalInput")
    # w2 d-chunk major: [e, i, dcb, hc, dd] = W2[e, dcb*128+dd, hc*128+i]
    w2t = nc.dram_tensor("w2t", [EPC, 128, DC, HC, 128], MMDT,
                         kind="ExternalInput")
    # combined biases: per slot, HC cols of b1 then DC cols of b2
    bct = nc.dram_tensor("bct", [128, EPC * BCOL], F32, kind="ExternalInput")

    GELU = mybir.ActivationFunctionType.Gelu

    with tile.TileContext(nc) as tc:
        with (
            tc.tile_pool(name="xp", bufs=1) as xp,
            tc.tile_pool(name="wp", bufs=1) as wp,
            tc.tile_pool(name="gp", bufs=2) as gp,
            tc.tile_pool(name="yp", bufs=3) as yp,
            tc.tile_pool(name="bp", bufs=1) as bp,
            tc.tile_pool(name="pm", bufs=6, space="PSUM") as pm,
            tc.tile_pool(name="p2", bufs=2, space="PSUM") as p2,
        ):
            # ---- phase 0: PE clock warmup. The PE ramps 0.65 -> 1.2 ->
            # 2.4 GHz with ~5us of continuous execution; these matmuls have
            # no DMA deps and run during the otherwise-idle head so real
            # matmuls start near full clock.
            wu = wp.tile([128, 640], MMDT, tag="wu", name="wu")
            nc.vector.memset(wu[:, :], 0.0)
            for i in range(NWARM):
                pw = p2.tile([128, 512], F32, tag="p2", name=f"pwu_{i}")
                nc.tensor.matmul(pw[:, :], wu[:, 0:128], wu[:, 128:640],
                                 start=True, stop=True)

            # ---- phase 1: issue ALL input DMAs. No compute-dependent wait
            # ever enters any HWDGE ring. HBM bandwidth is SHARED across the
            # rings (~450 GB/s/core aggregate), so later-needed tensors must
            # ride BEHIND earlier-needed ones on the same ring rather than
            # on a parallel ring (parallel rings steal bandwidth from the
            # critical path — measured +11us when slot1 weights ran on
            # their own ring during the slot0 window).
            # SP ring:  w1s0 pieces | w2s0 pieces | w1s1 halves (consume order)
            # ACT ring: x0 pieces | x1 | w2s1, then y outputs
            # POOL ring: biases (tiny), then tail y pieces
            w1s_t, w2s_t, xcs_t = [], [], []
            for s in range(EPC):
                w1s_t.append(wp.tile([128, DC, H], MMDT, tag=f"w1_{s}",
                                     name=f"w1s_{s}"))
                w2s_t.append(wp.tile([128, DC, HC, 128], MMDT, tag=f"w2_{s}",
                                     name=f"w2s_{s}"))
                xcs_t.append([xp.tile([128, DC, cw], MMDT, tag=f"x_{s}_{ci}",
                                      name=f"xc_{s}_{ci}")
                              for ci, (c0, cw) in enumerate(slot_chunks[s])])

            # slot0 w1 per-d-chunk pieces ALTERNATED across the SP and POOL
            # rings (a single ring moves small-elem pieces at only ~100-150
            # GB/s; two in parallel keep up with MM1's consumption), then
            # w2s0 halves, then w1s1 halves — strictly in consume order.
            bc = bp.tile([128, EPC * BCOL], F32, tag="bc", name="bc")
            nc.sync.dma_start(w1s_t[0][:, 0:1], w1t.ap()[0, :, 0:1])
            nc.gpsimd.dma_start(w1s_t[0][:, 1:2], w1t.ap()[0, :, 1:2])
            nc.sync.dma_start(w1s_t[0][:, 2:3], w1t.ap()[0, :, 2:3])
            nc.gpsimd.dma_start(bc[:, :], bct.ap())
            nc.sync.dma_start(w1s_t[0][:, 4:5], w1t.ap()[0, :, 4:5])
            nc.gpsimd.dma_start(w1s_t[0][:, 3:4], w1t.ap()[0, :, 3:4])
            nc.gpsimd.dma_start(w1s_t[0][:, 5:6], w1t.ap()[0, :, 5:6])
            nc.sync.dma_start(w2s_t[0][:, 0:3], w2t.ap()[0, :, 0:3])
            nc.gpsimd.dma_start(w2s_t[0][:, 3:6], w2t.ap()[0, :, 3:6])
            nc.sync.dma_start(w1s_t[1][:, 0:3], w1t.ap()[1, :, 0:3])
            nc.gpsimd.dma_start(w1s_t[1][:, 3:6], w1t.ap()[1, :, 3:6])

            # ACT ring: x slot0 in 4 pieces matching MM1's dc-outer
            # consumption.
            for ci, (c0, cw) in enumerate(slot_chunks[0]):
                for dl, dh in ((0, 1), (1, 2), (2, 4), (4, 6)):
                    nc.scalar.dma_start(xcs_t[0][ci][:, dl:dh, :],
                                        xts_d[0].ap()[:, dl:dh, c0:c0 + cw])
            # x1 and w2s1 are needed LAST — but a ring pulls as soon as its
            # descriptors are queued, and aggregate DMA bandwidth is shared
            # (~320 GB/s/core), so issuing them now would starve the slot0
            # pieces. Gate them on the DVE ring behind a 1-element read of
            # the final x0 piece: their descriptors enter the ring only
            # once the critical slot0 x is fully delivered.
            xgate = gp.tile([128, 1], F32, tag="xgate", name="xgate")
            nc.gpsimd.tensor_scalar_add(
                xgate[:, :], xcs_t[0][-1][:, DC - 1:DC, 0:1], 0.0)
            for ci, (c0, cw) in enumerate(slot_chunks[1]):
                nc.gpsimd.dma_start(xcs_t[1][ci][:, :, :],
                                    xts_d[1].ap()[:, :, c0:c0 + cw])
            nc.gpsimd.dma_start(w2s_t[1][:, :, :], w2t.ap()[1])

            # Preload the scalar engine's GELU table (ACT_TABLE_LOAD costs
            # ~1.3us on first use; do it in the DMA-wait head, after the
            # x-piece issues so it doesn't delay them).
            gwu = gp.tile([128, 1], F32, tag="gwu", name="gwu")
            nc.scalar.activation(gwu[:, :], wu[:, 0:1], GELU, bias=0.0,
                                 scale=1.0)

            # ---- phase 2: compute
            for s in range(EPC):
                chunks = slot_chunks[s]
                w1s, w2s, xcs = w1s_t[s], w2s_t[s], xcs_t[s]
                last_slot = (s == EPC - 1)
                for ci, (c0, cw) in enumerate(chunks):
                    xc = xcs[ci]
                    last_chunk = last_slot and (ci == len(chunks) - 1)
                    streamed = (s == 0 and ci == 0)
                    gc = gp.tile([128, HC, 512], MMDT, tag="g",
                                 name=f"gc_{s}_{ci}")
                    if streamed:
                        # MM1, d-chunk outer: 6 live PSUM banks, streams as
                        # the per-d-chunk w1/x pieces arrive.
                        ms = [pm.tile([128, 512], F32, tag="m",
                                      name=f"m_{s}_{ci}_{hc}")
                              for hc in range(HC)]
                        for dc in range(DC):
                            for hc in range(HC):
                                nc.tensor.matmul(
                                    ms[hc][:, :cw],
                                    w1s[:, dc, hc * 128:(hc + 1) * 128],
                                    xc[:, dc, :],
                                    start=(dc == 0), stop=(dc == DC - 1),
                                )
                        for hc in range(HC):
                            nc.scalar.activation(
                                gc[:, hc, :cw], ms[hc][:, :cw], GELU,
                                bias=bc[:, s * BCOL + hc:s * BCOL + hc + 1],
                                scale=1.0)
                    else:
                        # Weights already resident: h-chunk outer, each PSUM
                        # completes (and GELUs) early so MM2 never waits on
                        # the activation at the slot transition.
                        for hc in range(HC):
                            mh = pm.tile([128, 512], F32, tag="m",
                                         name=f"m_{s}_{ci}_{hc}")
                            for dc in range(DC):
                                nc.tensor.matmul(
                                    mh[:, :cw],
                                    w1s[:, dc, hc * 128:(hc + 1) * 128],
                                    xc[:, dc, :],
                                    start=(dc == 0), stop=(dc == DC - 1),
                                )
                            nc.scalar.activation(
                                gc[:, hc, :cw], mh[:, :cw], GELU,
                                bias=bc[:, s * BCOL + hc:s * BCOL + hc + 1],
                                scale=1.0)
                    # MM2: accumulate over h into 2 rotating banks; outputs
                    # grouped 3 d-chunks per DMA, except the very last
                    # group which flushes per-d-chunk on alternating rings
                    # so the tail pipeline drains early.
                    for g2 in range(2):
                        dl, dh = 3 * g2, 3 * (g2 + 1)
                        split_out = last_chunk and g2 == 1
                        yc = yp.tile([128, 3, cw], F16, tag="y",
                                     name=f"yc_{s}_{ci}_{g2}")
                        for dc in range(dl, dh):
                            ps2 = p2.tile([128, 512], F32, tag="p2",
                                          name=f"p2_{s}_{ci}_{dc}")
                            for hc in range(HC):
                                nc.tensor.matmul(
                                    ps2[:, :cw],
                                    w2s[:, dc, hc, :],
                                    gc[:, hc, :cw],
                                    start=(hc == 0), stop=(hc == HC - 1),
                                )
                            nc.vector.tensor_scalar_add(
                                yc[:, dc - dl, :], ps2[:, :cw],
                                bc[:, s * BCOL + HC + dc:
                                   s * BCOL + HC + dc + 1])
                            if split_out:
                                eng = (nc.scalar, nc.scalar, nc.gpsimd)[dc - dl]
                                eng.dma_start(
                                    yts_d[s].ap()[:, dc, c0:c0 + cw],
                                    yc[:, dc - dl, :])
                        if not split_out:
                            nc.scalar.dma_start(
                                yts_d[s].ap()[:, dl:dh, c0:c0 + cw],
                                yc[:, :, :])

    return _split_multi_waits(nc)


_NC_CACHE = {}


def _get_nc(C0, C1):
    key = (C0, C1, MM_DTYPE, NWARM)
    nc = _NC_CACHE.get(key)
    if nc is None:
        nc = _build(C0, C1)
        _NC_CACHE[key] = nc
    return nc


def _cap(n):
    return int(max(64, -(-int(n) // 32) * 32))


def kernel(x, W1, b1, W2, b2, Wg, bg):
    x = np.ascontiguousarray(np.asarray(x, dtype=np.float32))
    W1 = np.asarray(W1, dtype=np.float32)
    b1 = np.asarray(b1, dtype=np.float32)
    W2 = np.asarray(W2, dtype=np.float32)
    b2 = np.asarray(b2, dtype=np.float32)
    Wg = np.asarray(Wg, dtype=np.float32)
    bg = np.asarray(bg, dtype=np.float32)

    B, N, Dx = x.shape
    assert Dx == D and W1.shape == (E, H, D)
    T = B * N
    t = x.reshape(T, D)

    # --- gate / dispatch (host): this decides the sharding ---
    logits = t @ Wg.T + bg
    idx = np.argmax(logits, axis=1)

    counts = np.bincount(idx, minlength=E)
    # slot 0 <- 8 largest experts, slot 1 <- 8 smallest
    order = np.argsort(-counts, kind="stable")
    slot_experts = [order[:NCORES], order[NCORES:]]
    C0 = _cap(counts[slot_experts[0]].max())
    C1 = _cap(counts[slot_experts[1]].max())
    caps = [C0, C1]
    nc = _get_nc(C0, C1)
    _, npdt = _mm_dt()

    tok_ids = [np.nonzero(idx == e)[0] for e in range(E)]

    # --- host-side layout prep ---
    t_mm = t.astype(npdt)
    # w1t[e, i, dc, h] = W1[e, h, dc*128+i] (partition-major, chunk, col)
    w1t_all = np.ascontiguousarray(
        W1.astype(npdt).transpose(0, 2, 1).reshape(E, DC, 128, H)
        .transpose(0, 2, 1, 3))
    # w2t[e, i, dcb, hc, dd] = W2[e, dcb*128+dd, hc*128+i]
    w2t_all = np.ascontiguousarray(
        W2.astype(npdt).reshape(E, DC, 128, HC, 128).transpose(0, 4, 1, 3, 2))
    # bct[i, s*BCOL + hc] = b1[e_s, hc*128+i]; [..., HC + dc] = b2[e_s, ...]
    b1c_all = b1.reshape(E, HC, 128).transpose(0, 2, 1)
    b2c_all = b2.reshape(E, DC, 128).transpose(0, 2, 1)

    in_maps = []
    for c in range(NCORES):
        experts = [int(slot_experts[s][c]) for s in range(EPC)]
        bct = np.empty((128, EPC * BCOL), np.float32)
        for s in range(EPC):
            e = experts[s]
            bct[:, s * BCOL:s * BCOL + HC] = b1c_all[e]
            bct[:, s * BCOL + HC:(s + 1) * BCOL] = b2c_all[e]
        m = {
            "w1t": np.ascontiguousarray(w1t_all[experts]),
            "w2t": np.ascontiguousarray(w2t_all[experts]),
            "bct": bct,
        }
        for s in range(EPC):
            C = caps[s]
            xts = np.zeros((128, DC, C), npdt)
            ids = tok_ids[experts[s]]
            n = len(ids)
            if n:
                xts[:, :, :n] = (
                    t_mm[ids].T.reshape(DC, 128, n).transpose(1, 0, 2))
            m[f"xt{s}"] = xts
        in_maps.append(m)

    res = run_bass_kernel_spmd(nc, in_maps, core_ids=list(range(NCORES)))

    out = np.empty((T, D), np.float32)
    for c in range(NCORES):
        for s in range(EPC):
            e = int(slot_experts[s][c])
            ids = tok_ids[e]
            n = len(ids)
            if n:
                yt = res.results[c][f"yt{s}"]  # [128, DC, C] f16
                out[ids] = (yt.transpose(1, 0, 2).reshape(D, caps[s])[:, :n]
                            .astype(np.float32).T)
    return out.reshape(B, N, D)
